# revision 12
# baseline (speedup 1.0000x reference)
"""Trainium2 Bass kernel for nn_BandPassFilter (filtfilt FIR bank).

Math: composing the filtfilt forward+backward passes, each band's combined
filter is the autocorrelation c[n] = corr(w[n], w[n]) of length 2K-1 = 1537
applied to the odd-extended signal xext (length 9728):

    out[b, n, t] = sum_{k=0}^{1536} c[n, k] * xext[b, t + k]

Mixed-precision tap-split (tolerance is 2e-2; e4m3 noise is weighted by the
filter energy profile, which is concentrated at the center tap / triangular;
dropping any tap subset costs sqrt(energy share) so all 1537 taps are kept):
  - taps are processed in 13 chunks of 128 on a (k + m2) grid, m2 in [0,32)
  - chunks 5,6 (~65% of filter energy): fp16 matmuls (1 cyc/row)
  - chunks 4,7: fp8e4m3 DoubleRow, k-tile pair = (c_hi, c_lo) so c is at
    ~8-bit precision (x stays e4m3); chunk 12 (corner triangle) likewise,
    with a stride-0 lhsT k-tile dim
  - chunks (0,1),(2,3),(8,9),(10,11): plain fp8e4m3 DoubleRow pairs
    (256-deep contraction at 0.5 cyc/row = 4x fp16 rate)
  Simulated end-to-end worst-phase rel err: ~1.45e-2 < 2e-2.

Layout per core (8 batch rows):
  - xq[p, r, q] = xext[r, 32 q + p]: a 4x-overlapping column view (Q=32)
    so chunk j's stationary slice starts at q = 128 h + 4 j; the DoubleRow
    k-tile dim uses a raw overlapping AP (stride 4 or 12 in q).
  - moving cb blocks (host-precomputed Toeplitz): cb[p, ., n, m2] =
    c[n, 128 j + p - m2], 32-wide in m2 -> small cb (~1.3 MB/core).
  - psum chain per (row, half, band-group): out[f'(128), n, m2] accumulates
    8 matmuls; drained (scaled) to an fp16 collector, one 1.3KB-run DMA
    per (row, half).

Sharding: data-parallel over batch, 8 rows per NeuronCore, kernels
replicated.
"""
import numpy as np
import ml_dtypes

B, L, NB, K = 64, 8192, 20, 769
KC = 2 * K - 1        # 1537 combined filter length
PAD = K - 1           # 768
LE = L + 2 * PAD      # 9728 = 32 * 304
W = 32                # m2 tile width
QC = LE // W          # 304 q-columns
NCORES = 8
RPC = B // NCORES     # 8 rows per core
NCH = 13              # 128-tap chunks
GROUPS = [(0, 8), (8, 16), (16, 20)]
PLAIN_PAIRS = [(0, 1), (2, 3), (8, 9), (10, 11)]
P_PAIR = (4, 7)       # c hi/lo DoubleRow chunks
S_CHUNK = 12          # corner chunk, c hi/lo with stride-0 x k-tile
F_CHUNKS = [5, 6]     # fp16 chunks
XS = 16.0             # x scale into e4m3 range
CS = 32768.0          # c scale into e4m3 range
E4 = ml_dtypes.float8_e4m3

_CACHE = {}


def _program():
    import concourse.bass as bass
    import concourse.bacc as bacc
    import concourse.tile as tile
    from concourse import mybir

    f32 = mybir.dt.float32
    f16 = mybir.dt.float16
    f8 = mybir.dt.float8e4
    DR = mybir.MatmulPerfMode.DoubleRow
    nc = bacc.Bacc()

    xq8_d = nc.dram_tensor("xq8", [128, RPC, 2, 6, 2, 128], f8,
                           kind="ExternalInput")
    xq16_d = nc.dram_tensor("xq16", [128, RPC, QC], f16, kind="ExternalInput")
    cb8_d, cb16_d = [], []
    for gi, (n0, n1) in enumerate(GROUPS):
        nn = n1 - n0
        cb8_d.append(nc.dram_tensor(f"cb8_{gi}", [128, 7, 2, nn, W], f8,
                                    kind="ExternalInput"))
        cb16_d.append(nc.dram_tensor(f"cb16_{gi}", [128, 2, nn, W], f16,
                                     kind="ExternalInput"))
    out_d = nc.dram_tensor("out", [RPC, 2, 128, NB, W], f16,
                           kind="ExternalOutput")

    out_v = out_d[:].rearrange("r h f n m -> r f h n m")
    with tile.TileContext(nc) as tc:
        with (
            tc.tile_pool(name="xqp", bufs=1) as xqp,
            tc.tile_pool(name="cbp", bufs=3) as cbp,
            tc.tile_pool(name="cfp", bufs=3) as cfp,
            tc.tile_pool(name="colp", bufs=1) as colp,
            tc.tile_pool(name="wzp", bufs=1) as wzp,
            tc.tile_pool(name="psp", bufs=8, space=bass.MemorySpace.PSUM) as psp,
        ):
            xq8_t = xqp.tile([128, RPC, 2, 6, 2, 128], f8)
            xq16_t = xqp.tile([128, RPC, QC], f16)
            cb8_t = [cbp.tile([128, 7, 2, n1 - n0, W], f8, name=f"cb8t{gi}")
                     for gi, (n0, n1) in enumerate(GROUPS)]
            cb16_t = [cfp.tile([128, 2, n1 - n0, W], f16, name=f"cb16t{gi}")
                      for gi, (n0, n1) in enumerate(GROUPS)]

            # prologue, ordered for pass order [g2, g0, g1]: pass g2 only
            # needs xq + the small cb[2] blocks, so the PE starts ~1.8us in;
            # cb[0]/cb[1] stream during the g2/g0 passes
            nc.sync.dma_start(xq8_t[:, 0:1], xq8_d[:, 0:1])
            nc.scalar.dma_start(cb16_t[2][:], cb16_d[2][:])
            nc.sync.dma_start(cb8_t[2][:], cb8_d[2][:])
            nc.scalar.dma_start(xq16_t[:, 0:1], xq16_d[:, 0:1])
            for r in range(1, RPC):
                nc.sync.dma_start(xq8_t[:, r:r + 1], xq8_d[:, r:r + 1])
            nc.scalar.dma_start(xq16_t[:, 1:], xq16_d[:, 1:])
            nc.scalar.dma_start(cb16_t[0][:], cb16_d[0][:])
            nc.sync.dma_start(cb8_t[0][:], cb8_d[0][:])
            nc.scalar.dma_start(cb8_t[1][:], cb8_d[1][:])
            nc.scalar.dma_start(cb16_t[1][:], cb16_d[1][:])

            # warm the PE p-state during the startup DMA window (small
            # memset so warmups can begin almost immediately)
            wz = wzp.tile([128, 64], f16)
            nc.vector.memset(wz[:], 0.0)
            wps = psp.tile([128, 512], f32, tag="ps")
            for _ in range(14):
                nc.tensor.matmul(wps[:64, :64], wz[:], wz[:],
                                 start=True, stop=True)

            def lhs8(r, h, slot):
                return xq8_t[:, r, h, slot]

            # group-major: 16 chains per group pass, so pass 0 only needs
            # cb[0] resident; collectors persist across passes, stores are
            # split (bands 0:16 on the idle SWDGE ring during the g1 pass,
            # the tail bands on HWDGE during the g2 pass)
            cols = [colp.tile([128, 2, NB, W], f16, name=f"col{r}")
                    for r in range(RPC)]
            dix = 0
            for pi, gi in enumerate([2, 0, 1]):
                n0, n1 = GROUPS[gi]
                nn = n1 - n0
                for r in range(RPC):
                    for h in range(2):
                        col = cols[r]
                        ps = psp.tile([128, 512], f32, tag="ps")
                        out_ap = ps[:, :nn * W]
                        for si in range(4):     # plain DR chunk pairs
                            nc.tensor.matmul(
                                out_ap, lhs8(r, h, si),
                                cb8_t[gi][:, si],
                                start=(si == 0), stop=False, perf_mode=DR)
                        for si in (4, 5):   # c_hi then c_lo, chunks (4,7)
                            nc.tensor.matmul(
                                out_ap, lhs8(r, h, 4),
                                cb8_t[gi][:, si],
                                start=False, stop=False, perf_mode=DR)
                        # corner chunk 12: (c_hi, c_lo), x duplicated in
                        # both k-tile slots host-side
                        nc.tensor.matmul(
                            out_ap, lhs8(r, h, 5),
                            cb8_t[gi][:, 6],
                            start=False, stop=False, perf_mode=DR)
                        for fi, j in enumerate(F_CHUNKS):
                            q0 = h * 128 + 4 * j
                            nc.tensor.matmul(
                                out_ap, xq16_t[:, r, q0:q0 + 128],
                                cb16_t[gi][:, fi],
                                start=False, stop=(fi == len(F_CHUNKS) - 1))
                        pin = out_ap.rearrange("p (n m) -> p n m", n=nn)
                        nc.scalar.mul(col[:, h, n0:n1], pin, 1.0 / (XS * CS))
                        dix += 1
                        if h == 1 and pi == 2:
                            eng = nc.scalar if r % 2 == 0 else nc.sync
                            eng.dma_start(out_v[r], col[:])
    nc.compile()
    return nc


def _prep(x, kernels):
    xs = np.asarray(x)[:, 0, :].astype(np.float32)
    w = np.asarray(kernels).astype(np.float64)
    xe = np.concatenate(
        [-xs[:, PAD:0:-1], xs, -xs[:, L - 2:L - 2 - PAD:-1]], axis=1)
    # per-core xq[p, r, q] = xe[r, 32 q + p] (overlapping columns)
    idx = (W * np.arange(QC))[None, :] + np.arange(128)[:, None]  # [128, QC]
    idx = np.minimum(idx, LE - 1)  # tail columns are never read past LE
    jmap = np.array(PLAIN_PAIRS + [list(P_PAIR), [S_CHUNK, S_CHUNK]])
    qidx = (128 * np.arange(2)[:, None, None, None]
            + 4 * jmap[None, :, :, None]
            + np.arange(128)[None, None, None, :])   # [2, 6, 2, 128]
    xq16_cores, xq8_cores = [], []
    for cc in range(NCORES):
        xv = xe[cc * RPC:(cc + 1) * RPC] * XS        # [RPC, LE]
        xq = xv[:, idx].transpose(1, 0, 2)           # [128, RPC, QC]
        xq16_cores.append(np.ascontiguousarray(xq.astype(np.float16)))
        xq8_cores.append(np.ascontiguousarray(
            xq[:, :, qidx].astype(E4)))              # [128,RPC,2,6,2,128]

    c = np.stack([np.correlate(w[n], w[n], "full") for n in range(NB)])
    c = c * CS
    c16 = c.astype(np.float16)
    c8 = c.astype(E4)
    clo8 = (c - c8.astype(np.float64)).astype(E4)

    # Toeplitz blocks: blk[p, n, m2] = cv[n, 128 j + p - m2], masked to k>=0
    kidx = (np.arange(128)[:, None, None] - np.arange(W)[None, None, :])
    def blk(cv, j):
        ki = 128 * j + kidx                           # [128, 1, W]
        valid = (ki >= 0) & (ki < KC)
        ki = np.clip(ki, 0, KC - 1)
        return np.where(valid, cv.astype(np.float64)[:, ki[:, 0, :]]
                        .transpose(1, 0, 2), 0.0)     # [128, n, W]

    cb8_list, cb16_list = [], []
    for (n0, n1) in GROUPS:
        nn = n1 - n0
        b8 = np.zeros((128, 7, 2, nn, W), np.float64)
        for si, (ja, jb) in enumerate(PLAIN_PAIRS):
            b8[:, si, 0] = blk(c8[n0:n1], ja)
            b8[:, si, 1] = blk(c8[n0:n1], jb)
        b8[:, 4, 0] = blk(c8[n0:n1], P_PAIR[0])
        b8[:, 4, 1] = blk(c8[n0:n1], P_PAIR[1])
        b8[:, 5, 0] = blk(clo8[n0:n1], P_PAIR[0])
        b8[:, 5, 1] = blk(clo8[n0:n1], P_PAIR[1])
        b8[:, 6, 0] = blk(c8[n0:n1], S_CHUNK)
        b8[:, 6, 1] = blk(clo8[n0:n1], S_CHUNK)
        b16 = np.zeros((128, 2, nn, W), np.float64)
        for fi, j in enumerate(F_CHUNKS):
            b16[:, fi] = blk(c16[n0:n1], j)
        cb8_list.append(np.ascontiguousarray(b8.astype(E4)))
        cb16_list.append(np.ascontiguousarray(b16.astype(np.float16)))

    in_maps = []
    for cc in range(NCORES):
        m = {"xq8": xq8_cores[cc], "xq16": xq16_cores[cc]}
        for gi in range(len(GROUPS)):
            m[f"cb8_{gi}"] = cb8_list[gi]
            m[f"cb16_{gi}"] = cb16_list[gi]
        in_maps.append(m)
    return in_maps


def _assemble(res_list):
    # out[r, h, f', n, m2] fp16 -> [B, 1, NB, L] fp32
    outs = []
    for cc in range(NCORES):
        o = np.asarray(res_list[cc]["out"]).astype(np.float32)
        o = o.transpose(0, 3, 1, 2, 4).reshape(RPC, NB, L)
        outs.append(o)
    return np.concatenate(outs, axis=0)[:, None]


def kernel(x, kernels):
    from concourse.bass_utils import run_bass_kernel_spmd

    if "nc" not in _CACHE:
        _CACHE["nc"] = _program()
    nc = _CACHE["nc"]
    in_maps = _prep(x, kernels)
    res = run_bass_kernel_spmd(nc, in_maps, core_ids=list(range(NCORES)))
    return _assemble(res.results)
